# revision 2
# baseline (speedup 1.0000x reference)
"""Expert-parallel MoE FFN (ChronosMOEFeedForward) for 8 Trainium2 cores.

Strategy (sharding_hint: expert-parallel):
  - Router (softmax + top-2 over E=16 experts) computed on host in fp32 —
    top-k decisions must match the fp32 reference's ordering, and the router
    GEMM is ~0.1% of total FLOPs.
  - The 16 experts are sharded 2-per-core across 8 cores. Tokens routed to
    each expert are gathered on host (the "all-to-all dispatch"), padded to a
    fixed capacity C=512, and shipped transposed as [H, C] so the device GEMM
    chain needs no on-device transposes.
  - Per core the device computes, per expert e:
        gT = Wg[e].T @ XeT            [I, C]   (bf16 inputs, fp32 PSUM accum)
        sT = silu(gT)                 [I, C]   (fused SiLU on ScalarE)
        uT = Wu[e].T @ XeT            [I, C]
        aT = sT * uT * w_bcast        [I, C]   (combine weight broadcast)
        yT = Wd[e].T @ aT             [H, C]   (tokens stay on the free dim)
  - Host scatters each expert's y rows back to the owning tokens ("combine").
    A token's two expert contributions land in two disjoint slot arrays
    (top-1 slot, top-2 slot), so the combine is collision-free fancy
    indexing plus one add — no np.add.at.
  - Expert loads are ~N(512, 22); the uniform C=512 capacity covers the mean
    load exactly and keeps every matmul at the full 512-column PSUM bank
    width. The ~1-2% of tokens on experts whose load exceeds 512 fall back
    to an exact fp32 numpy path on the host.

The schedule is k-contiguous per (mat, m-tile): 16 (or 8) accumulating
matmuls fill one PSUM bank back-to-back, rotating over all 8 banks, so the
PE never waits on evictions (ScalarE does the SiLU, VectorE the muls and
PSUM->SBUF copies). Weight/x DMAs for expert e+1 stream while expert e
computes (double-buffered gw/uw and xg pools).

The dense reference formulation computes all 16 experts for every token;
routed top-2 computes only 2 — an 8x FLOP reduction, plus bf16 matmuls with
fp32 PSUM accumulation.
"""

import numpy as np
import ml_dtypes

import concourse.mybir as mybir
import concourse.tile as tile
from concourse import bacc
from concourse.bass_utils import run_bass_kernel_spmd

# Problem shapes (hardcoded per contract).
H = 2048        # hidden size
I = 1024        # moe intermediate size
E = 16          # num experts
TOPK = 2
B, S = 4, 1024
T = B * S       # 4096 tokens
N_CORES = 8
EPC = E // N_CORES  # experts per core = 2
# Uniform per-expert token capacity: one full PSUM bank of columns. Expert
# loads are ~N(512, 22), so 512 covers the mean; overflow tokens (~1-2%)
# take the exact numpy path on host.
CAPS = (512, 512)
C = CAPS[0]

BF16 = ml_dtypes.bfloat16

KT_H = H // 128   # 16 k-tiles over H (phase A contraction)
MT_I = I // 128   # 8 m-tiles over I
KT_I = I // 128   # 8 k-tiles over I (phase B contraction)
MT_H = H // 128   # 16 m-tiles over H

_CACHE = {}


def _build_nc(caps=CAPS, loop_r=None, internal=False):
    """Build the per-core Bass module (SPMD: all cores run this program).

    loop_r/internal are for the timing harness only: Internal DRAM I/O (no
    host transfers) with the body repeated 2*loop_r times on-device (two
    body copies per hardware-loop iteration halve the ~2-5us Tile back-edge
    cost in the measured slope).
    """
    import contextlib

    nc = bacc.Bacc(None, target_bir_lowering=False)
    f32 = mybir.dt.float32
    bf16 = mybir.dt.bfloat16

    if internal:
        xg = nc.dram_tensor("xg", [EPC, H, C], bf16)
        gww = nc.dram_tensor("gww", [EPC, H, I], bf16)
        uww = nc.dram_tensor("uww", [EPC, H, I], bf16)
        wdp = nc.dram_tensor("wdp", [EPC, I, H], bf16)
        wtv = nc.dram_tensor("wtv", [EPC, C], f32)
        y = nc.dram_tensor("y", [EPC, H, C], bf16)
        done = nc.declare_dram_parameter("done", [1, 1], f32, isOutput=True)
    else:
        xg = nc.declare_dram_parameter("xg", [EPC, H, C], bf16, isOutput=False)
        gww = nc.declare_dram_parameter("gww", [EPC, H, I], bf16, isOutput=False)
        uww = nc.declare_dram_parameter("uww", [EPC, H, I], bf16, isOutput=False)
        wdp = nc.declare_dram_parameter("wdp", [EPC, I, H], bf16, isOutput=False)
        wtv = nc.declare_dram_parameter("wtv", [EPC, C], f32, isOutput=False)
        y = nc.declare_dram_parameter("y", [EPC, H, C], bf16, isOutput=True)

    with tile.TileContext(nc) as tc:
        with (
            tc.tile_pool(name="wpool", bufs=2) as wpool,   # gw/uw share slots
            tc.tile_pool(name="xpool", bufs=2) as xpool,
            tc.tile_pool(name="wdpool", bufs=1) as wdpool,
            tc.tile_pool(name="apool", bufs=1) as apool,
            tc.tile_pool(name="small", bufs=2) as small,
            tc.tile_pool(name="yp", bufs=4) as yp,
            tc.tile_pool(name="ps", bufs=8, space="PSUM") as ps,
        ):
            const = small.tile([1, 128], f32, tag="ones")
            nc.any.memset(const[:], 1.0)

            loop_cm = (
                tc.For_i(0, loop_r, 1) if loop_r else contextlib.nullcontext()
            )
            with loop_cm:
                for _rep in range(2 if loop_r else 1):
                    _emit_body(nc, tc, xg, gww, uww, wdp, wtv, y, const,
                               wpool, xpool, wdpool, apool, small, yp, ps)

            if internal:
                dn = small.tile([1, 1], f32, tag="done")
                nc.any.memset(dn[:], 1.0)
                nc.sync.dma_start(out=done[:], in_=dn[:])

    nc.compile()
    return nc


def _emit_body(nc, tc, xg, gww, uww, wdp, wtv, y, const,
               wpool, xpool, wdpool, apool, small, yp, ps):
    f32 = mybir.dt.float32
    bf16 = mybir.dt.bfloat16
    for e in range(EPC):
        # DMA issue order = need order: combine weights (tiny, feeds the
        # wbc broadcast placed first), gw+xg k-tiles interleaved (phase A
        # ramp), then wu, then wd (phase B).
        wt_sb = small.tile([1, C], f32, tag="wt")
        nc.sync.dma_start(out=wt_sb[:], in_=wtv[e][None, :])
        gw_sb = wpool.tile([128, KT_H, I], bf16, tag="guw")
        xg_sb = xpool.tile([128, KT_H, C], bf16, tag="xg")
        for ko in range(KT_H):
            nc.sync.dma_start(
                out=gw_sb[:, ko, :], in_=gww[e, ko * 128 : (ko + 1) * 128, :]
            )
            nc.sync.dma_start(
                out=xg_sb[:, ko, :], in_=xg[e, ko * 128 : (ko + 1) * 128, :]
            )
        uw_sb = wpool.tile([128, KT_H, I], bf16, tag="guw")
        for ko in range(KT_H):
            nc.sync.dma_start(
                out=uw_sb[:, ko, :], in_=uww[e, ko * 128 : (ko + 1) * 128, :]
            )
        wd_sb = wdpool.tile([128, KT_I, H], bf16, tag="wd")
        for ko in range(KT_I):
            nc.sync.dma_start(
                out=wd_sb[:, ko, :], in_=wdp[e, ko * 128 : (ko + 1) * 128, :]
            )

        sg_sb = apool.tile([128, MT_I, C], bf16, tag="sg")
        a_sb = apool.tile([128, MT_I, C], bf16, tag="a")
        wbc_sb = small.tile([128, C], bf16, tag="wbc")

        # broadcast combine weights across partitions via outer product
        # ones[128] x wt[C] -> wbc[128, C]; runs inside the initial DMA ramp
        # (wt is the first DMA issued) before phase A claims the bank
        pw = ps.tile([128, 512], f32, tag="ps", name="ps_wbc")
        nc.tensor.matmul(pw[:], lhsT=const[:], rhs=wt_sb[:], start=True,
                         stop=True)
        nc.vector.tensor_copy(wbc_sb[:], pw[:])

        # ---- phase A: gT then uT, k-contiguous per m-tile, one PSUM bank
        # per (mat, m) group rotating over the 8 banks
        for mat in range(2):  # 0: g (silu), 1: u (mul + combine weight)
            w_sb = gw_sb if mat == 0 else uw_sb
            for m in range(MT_I):
                pt = ps.tile([128, 512], f32, tag="ps", name=f"ps_{mat}_{m}")
                for k in range(KT_H):
                    nc.tensor.matmul(
                        pt[:],
                        lhsT=w_sb[:, k, m * 128 : (m + 1) * 128],
                        rhs=xg_sb[:, k, :],
                        start=(k == 0),
                        stop=(k == KT_H - 1),
                    )
                if mat == 0:
                    nc.scalar.activation(
                        sg_sb[:, m, :], pt[:], mybir.ActivationFunctionType.Silu
                    )
                else:
                    tmp = small.tile([128, C], bf16, tag="tmp")
                    nc.vector.tensor_mul(tmp[:], sg_sb[:, m, :], pt[:])
                    nc.vector.tensor_mul(a_sb[:, m, :], tmp[:], wbc_sb[:])

        # ---- phase B: yT = Wd.T @ a   [H, C] — tokens stay on the free dim
        for m in range(MT_H):
            pt = ps.tile([128, 512], f32, tag="ps", name=f"bps_{m}")
            for k in range(KT_I):
                nc.tensor.matmul(
                    pt[:],
                    lhsT=wd_sb[:, k, m * 128 : (m + 1) * 128],
                    rhs=a_sb[:, k, :],
                    start=(k == 0),
                    stop=(k == KT_I - 1),
                )
            yt = yp.tile([128, C], bf16, tag="ysb")
            nc.vector.tensor_copy(yt[:], pt[:])
            nc.sync.dma_start(
                out=y[e, m * 128 : (m + 1) * 128, :], in_=yt[:]
            )


def _route(xf, gate_w):
    """Top-2 routing, mirroring the fp32 reference semantics exactly."""
    logits = xf @ gate_w.T.astype(np.float32)          # [T, E]
    logits -= logits.max(axis=-1, keepdims=True)
    scores = np.exp(logits)
    scores /= scores.sum(axis=-1, keepdims=True)
    i1 = scores.argmax(axis=-1)
    s1 = scores[np.arange(T), i1]
    masked = scores.copy()
    masked[np.arange(T), i1] = -np.inf
    i2 = masked.argmax(axis=-1)
    s2 = scores[np.arange(T), i2]
    denom = s1 + s2 + 1e-20
    return i1, s1 / denom, i2, s2 / denom


def _expert_np(xrows, wts, wg_e, wu_e, wd_e):
    """Exact fp32 fallback for capacity-overflow tokens (rare)."""
    g = xrows @ wg_e
    u = xrows @ wu_e
    a = (g / (1.0 + np.exp(-g))) * u * wts[:, None]
    return a @ wd_e


def _pack(xf, gate_w, wg, wu, wd):
    """Route + gather + pack per-core device inputs.

    Experts are assigned to (core, slot) by load so each core gets one
    heavier and one lighter expert; with the uniform C=512 capacity this
    only balances the host-side overflow work. The assignment is pure
    host-side data placement — the SPMD program is identical on every core.
    """
    i1, w1, i2, w2 = _route(xf, gate_w)
    per_e = []
    for e in range(E):
        l1 = np.nonzero(i1 == e)[0]
        l2 = np.nonzero(i2 == e)[0]
        toks = np.concatenate([l1, l2])
        wts = np.concatenate([w1[l1], w2[l2]])
        ranks = np.concatenate(
            [np.zeros(len(l1), np.int8), np.ones(len(l2), np.int8)]
        )
        per_e.append((toks, ranks, wts))
    loads = [len(pe[0]) for pe in per_e]
    order = np.argsort([-n for n in loads], kind="stable")

    in_maps = []
    tok_lists = []
    for c in range(N_CORES):
        xgc = np.zeros((EPC, H, C), BF16)
        wtc = np.zeros((EPC, C), np.float32)
        core_toks = []
        experts = [int(order[c]), int(order[2 * N_CORES - 1 - c])]
        for j in range(EPC):
            e = experts[j]
            toks, ranks, wts = per_e[e]
            n_dev = min(len(toks), CAPS[j])
            xgc[j, :, :n_dev] = xf[toks[:n_dev]].T.astype(BF16)
            wtc[j, :n_dev] = wts[:n_dev]
            core_toks.append((toks, ranks, wts, n_dev, e))
        tok_lists.append(core_toks)
        in_maps.append(
            {
                "xg": xgc,
                "gww": wg[experts].astype(BF16),
                "uww": wu[experts].astype(BF16),
                "wdp": wd[experts].astype(BF16),
                "wtv": wtc,
            }
        )
    return in_maps, tok_lists, CAPS


def kernel(x, gate_w, wg, wu, wd):
    in_dtype = x.dtype
    xf = np.ascontiguousarray(x.reshape(T, H), dtype=np.float32)
    wg = np.asarray(wg, dtype=np.float32)
    wu = np.asarray(wu, dtype=np.float32)
    wd = np.asarray(wd, dtype=np.float32)

    in_maps, tok_lists, caps = _pack(xf, gate_w, wg, wu, wd)
    if caps not in _CACHE:
        _CACHE[caps] = _build_nc(caps)
    nc = _CACHE[caps]
    out1 = np.zeros((T, H), np.float32)
    out2 = np.zeros((T, H), np.float32)

    res = run_bass_kernel_spmd(nc, in_maps, core_ids=list(range(N_CORES)))
    _CACHE["last_in_maps"] = in_maps
    _CACHE["last_caps"] = caps
    _CACHE["nc"] = nc

    for c in range(N_CORES):
        yc = res.results[c]["y"].astype(np.float32)        # [EPC, H, C] (yT)
        for j in range(EPC):
            toks, ranks, wts, n_dev, e = tok_lists[c][j]
            yr = np.ascontiguousarray(yc[j, :, :n_dev].T)
            sel1 = ranks[:n_dev] == 0
            sel2 = ~sel1
            out1[toks[:n_dev][sel1]] = yr[sel1]
            out2[toks[:n_dev][sel2]] = yr[sel2]
            if len(toks) > n_dev:                          # capacity overflow
                extra = toks[n_dev:]
                yextra = _expert_np(xf[extra], wts[n_dev:], wg[e], wu[e], wd[e])
                r = ranks[n_dev:]
                out1[extra[r == 0]] = yextra[r == 0]
                out2[extra[r == 1]] = yextra[r == 1]

    out = (out1 + out2).reshape(B, S, H)
    return out.astype(in_dtype, copy=False)
